# revision 1
# baseline (speedup 1.0000x reference)
"""Trainium2 Bass kernel for ComplexMultiheadAttention.

Sharding: core c = b*4 + g (b = batch 0..1, g = head-group 0..3, 4 heads each).
Complex arithmetic folded into stacked real matmuls via host-side packing.

Design notes:
  * fp16 activations/weights everywhere the matmuls allow (same PE throughput
    as fp32r on TRN2, half the HBM/DMA/SBUF/upload bytes); exp probabilities
    and V stay bf16 (exp needs bf16 range without max-subtraction)
  * one fused [128,1024] exp per (mc, half) -- the score tile spans two PSUM
    banks, halving ACT instruction overhead in the attention phase
  * softmax row-sums as 4-way col-tiled ones-matmul chains (RS_MODE="ct"):
    the attention inner loop runs in 4-chunk blocks so the four M=32 rowsum
    matmuls (tile_position (0,32g), disjoint col-groups, one shared PSUM
    bank) issue back-to-back and overlap on the PE array; HW-probed exact
    (per-chain start=True; the has_written clear is per-partition). A
    1/32-ones matmul sums the four 32-row partials and broadcasts to 128
    rows for the normalizer.
  * all four weight slabs preloaded once and SBUF-resident
  * x packed per-phase so every DMA is >=2KB/partition contiguous
  * per-head fp16 AllGather (half the fp32 payload); gathered output kept
    SBUF-resident for the out-projection
  * timing-only path (ag_local) emulates the gather with SWDGE copies

Layouts (per core):
  xq/xk : [128, NT, KC, 512]  (p, n, k, c) = xstk[k*128+p, n*512+c]
  xv    : [128, KC, KC, 128]  (p, lc, k, c) = xstk[k*128+p, lc*128+c]
  w*    : [128, KC, 512]      (p, k, j) = Wstk[k*128+p, j]
  qs/ks : SBUF [128, HL, L]   per head h: rows 0:64 = q_r.T, 64:128 = q_i.T
  vs    : SBUF [128, KC, EL]  bf16 (pairs with bf16 exp in PV matmul)
  ex    : bf16 exp(scores.T) [key m (partitions), query l (free)]
  ot    : fp16 normalized attention out -> DRAM ag_in -> per-head AllGather
  og    : SBUF-resident [128, KC, L] fp16 copy of ag_out for out-proj
  y     : [512, 2048] fp32 slice of [y_r.T ; y_i.T]
"""

import os
import sys

for _p in ("/opt/trn_rl_repo",):
    if os.path.isdir(_p) and _p not in sys.path:
        sys.path.insert(0, _p)

import numpy as np

import concourse.bacc as bacc
import concourse.bass_isa as bass_isa
import concourse.mybir as mybir
import concourse.tile as tile

B, L, E, H = 2, 2048, 1024, 16
D = E // H          # 64
NCORES = 8
GROUPS = 4          # head-groups (tensor parallel inside a batch)
HL = H // GROUPS    # heads per core = 4
EL = HL * 2 * D     # stacked rows per core = 512
KC = 16             # 2048 / 128 contraction chunks
NT = L // 512       # 4 moving tiles over L
MT = EL // 128      # 4 output row tiles

F32 = mybir.dt.float32
F32R = mybir.dt.float32r
BF16 = mybir.dt.bfloat16
F16 = mybir.dt.float16
EXP = mybir.ActivationFunctionType.Exp
IDENT = mybir.ActivationFunctionType.Identity
MULT = mybir.AluOpType.mult
ADD = mybir.AluOpType.add


# number of per-body AllGathers: 4 = one per head (max overlap with
# attention compute), 1 = one bulk gather (min per-collective fixed cost).
# prep_in_maps' out-proj row permutation follows this choice.
AG_CHUNKS = 4

# rowsum mechanism: "mm" = full ones-matmul (baseline), "ct" = 4-way
# col-tiled matmul chains in one PSUM bank (needs per-element has_written
# semantics, probed on HW), "par" = DVE accumulate + GpSimd
# partition_all_reduce (no PE work at all)
RS_MODE = "ct"


def build_nc(repeat: int = 1, ag_local: bool = False, loop: int = 0):
    nc = bacc.Bacc("TRN2", target_bir_lowering=False, debug=False,
                   num_devices=NCORES)

    xq = nc.dram_tensor("xq", [128, NT, KC, 512], F16, kind="ExternalInput").ap()
    xk = nc.dram_tensor("xk", [128, NT, KC, 512], F16, kind="ExternalInput").ap()
    xv = nc.dram_tensor("xv", [128, KC, KC, 128], F16, kind="ExternalInput").ap()
    wq = nc.dram_tensor("wq", [128, KC, EL], F16, kind="ExternalInput").ap()
    wk = nc.dram_tensor("wk", [128, KC, EL], F16, kind="ExternalInput").ap()
    wv = nc.dram_tensor("wv", [128, KC, EL], F16, kind="ExternalInput").ap()
    wo = nc.dram_tensor("wo", [128, KC, EL], F16, kind="ExternalInput").ap()
    ones = nc.dram_tensor("ones", [128, 128], BF16, kind="ExternalInput").ap()
    ones32 = nc.dram_tensor("ones32", [128, 32], BF16, kind="ExternalInput").ap()
    comb = nc.dram_tensor("comb", [128, 128], F32R, kind="ExternalInput").ap()
    bq = nc.dram_tensor("bq", [128, MT], F32, kind="ExternalInput").ap()
    bk = nc.dram_tensor("bk", [128, MT], F32, kind="ExternalInput").ap()
    bo = nc.dram_tensor("bo", [128, MT], F32, kind="ExternalInput").ap()
    bv = nc.dram_tensor("bv", [128, EL], F32, kind="ExternalInput").ap()
    y = nc.dram_tensor("y", [EL, L], F32, kind="ExternalOutput").ap()

    rg = [[0, 1, 2, 3], [4, 5, 6, 7]]

    with tile.TileContext(nc) as tc:
        with tc.tile_pool(name="persist", bufs=1) as persist:
            ones_t = persist.tile([128, 128], BF16)
            nc.sync.dma_start(ones_t[:], ones[:])
            ones32_t = persist.tile([128, 32], BF16)
            nc.sync.dma_start(ones32_t[:], ones32[:])
            comb_t = persist.tile([128, 128], F32R)
            nc.sync.dma_start(comb_t[:], comb[:])
            ones_t = (ones_t, ones32_t, comb_t)
            bq_t = persist.tile([128, MT], F32)
            nc.sync.dma_start(bq_t[:], bq[:])
            bk_t = persist.tile([128, MT], F32)
            nc.sync.dma_start(bk_t[:], bk[:])
            bo_t = persist.tile([128, MT], F32)
            nc.sync.dma_start(bo_t[:], bo[:])
            bv_t = persist.tile([128, EL], F32)
            nc.sync.dma_start(bv_t[:], bv[:])
            w_t = {}
            for name, w_d in (("wq", wq), ("wk", wk), ("wv", wv), ("wo", wo)):
                w_t[name] = persist.tile([128, KC, EL], F16, name=f"w_{name}")
                for c in range(2):
                    nc.sync.dma_start(w_t[name][:, c * 8:(c + 1) * 8, :],
                                      w_d[:, c * 8:(c + 1) * 8, :])

            if loop:
                with tc.For_i(0, loop, 1):
                    _emit_body(nc, tc, 0, xq, xk, xv, w_t, y,
                               ones_t, bq_t, bk_t, bo_t, bv_t, rg,
                               ag_local=ag_local)
            else:
                for rep in range(repeat):
                    _emit_body(nc, tc, rep, xq, xk, xv, w_t, y,
                               ones_t, bq_t, bk_t, bo_t, bv_t, rg,
                               ag_local=ag_local)

    nc.compile()
    return nc


def _emit_body(nc, tc, rep, xq, xk, xv, w_t, y,
               ones_t, bq_t, bk_t, bo_t, bv_t, rg, ag_local=False):
    ag_in = nc.dram_tensor(f"ag_in_{rep}", [EL, L], F16).ap()
    ag_out = nc.dram_tensor(f"ag_out_{rep}", [GROUPS * EL, L], F16).ap()
    ag_in_v = ag_in.rearrange("(h p) l -> h p l", p=128)
    ag_out_v = ag_out.rearrange("(k p) l -> k p l", p=128)

    with tc.tile_pool(name="qkv_sb", bufs=1) as qkv_sb:
        qs_sb = qkv_sb.tile([128, HL, L], F16)
        ks_sb = qkv_sb.tile([128, HL, L], F16)
        vs_sb = qkv_sb.tile([128, KC, EL], BF16)

        def qk_phase(x_d, w, out_sb, bias_t):
            with tc.tile_pool(name="xp", bufs=2) as xp, \
                 tc.tile_pool(name="pp", bufs=8, space="PSUM") as pp:
                for n in range(NT):
                    ls = slice(n * 512, (n + 1) * 512)
                    accs = [pp.tile([128, 512], F32, name=f"qk_acc{m}",
                                    tag="qk_acc")
                            for m in range(MT)]
                    xt = xp.tile([128, KC, 512], F16, name="xqk")
                    nc.sync.dma_start(xt[:], x_d[:, n])
                    for k in range(KC):
                        for m in range(MT):
                            nc.tensor.matmul(
                                accs[m][:],
                                w[:, k, m * 128:(m + 1) * 128],
                                xt[:, k, :],
                                start=(k == 0), stop=(k == KC - 1))
                    for m in range(MT):
                        nc.scalar.activation(out_sb[:, m, ls], accs[m][:],
                                             IDENT, bias=bias_t[:, m:m + 1])

        # ---------------- Q / K projections ----------------
        qk_phase(xq, w_t["wq"], qs_sb, bq_t)
        qk_phase(xk, w_t["wk"], ks_sb, bk_t)

        # ---------------- V projection ----------------
        with tc.tile_pool(name="xp", bufs=3) as xp, \
             tc.tile_pool(name="pp", bufs=4, space="PSUM") as pp:
            for lc in range(KC):
                acc = pp.tile([128, EL], F32, name="v_acc")
                xt = xp.tile([128, KC, 128], F16, name="xv_t")
                nc.sync.dma_start(xt[:], xv[:, lc])
                for k in range(KC):
                    nc.tensor.matmul(acc[:], xt[:, k, :], w_t["wv"][:, k, :],
                                     start=(k == 0), stop=(k == KC - 1))
                nc.vector.tensor_tensor(vs_sb[:, lc, :], acc[:], bv_t[:], ADD)

        # ---------------- attention (per head) ----------------
        ones_full, ones32_t, comb_t = ones_t
        rs_bufs = 2 if RS_MODE != "par" else 0
        sc_bufs = 2 if RS_MODE != "par" else 3
        with tc.tile_pool(name="scp", bufs=sc_bufs, space="PSUM") as scp, \
             tc.tile_pool(name="pvp", bufs=2, space="PSUM") as pvp, \
             tc.tile_pool(name="rsp", bufs=max(1, rs_bufs),
                          space="PSUM") as rsp, \
             tc.tile_pool(name="ep", bufs=6) as ep, \
             tc.tile_pool(name="eac", bufs=2) as eac, \
             tc.tile_pool(name="otp", bufs=4) as otp:
            for h in range(HL):
                for half in range(2):
                    ns = (2 * half, 2 * half + 1)
                    pv2 = [pvp.tile([128, 512], F32, name=f"pv{j}", tag="pv")
                           for j in range(2)]
                    if RS_MODE != "par":
                        rs2 = [rsp.tile([128, 512], F32, name=f"rs{j}",
                                        tag="rs")
                               for j in range(2)]
                    else:
                        exacc = eac.tile([128, 1024], F32, name="exacc")
                    exs = {}
                    for blk in range(KC // 4):
                        # 4 mc-chunks per block: scores+exp+PV per chunk,
                        # then the 4 col-tiled rowsum MMs per j issued
                        # back-to-back so their col-groups overlap on the PE
                        for q in range(4):
                            mc = 4 * blk + q
                            ms = slice(mc * 128, (mc + 1) * 128)
                            ex = ep.tile([128, 1024], BF16, name="ex")
                            exs[q] = ex
                            sc = scp.tile([128, 1024], F32, name="sc",
                                          tag="sc")
                            for j, n in enumerate(ns):
                                ls = slice(n * 512, (n + 1) * 512)
                                js = slice(j * 512, (j + 1) * 512)
                                nc.tensor.matmul(sc[:, js], ks_sb[:, h, ms],
                                                 qs_sb[:, h, ls],
                                                 start=True, stop=True)
                            nc.scalar.activation(ex[:], sc[:], EXP,
                                                 scale=float(1.0 / np.sqrt(D)))
                            for j, n in enumerate(ns):
                                js = slice(j * 512, (j + 1) * 512)
                                nc.tensor.matmul(
                                    pv2[j][:],
                                    vs_sb[:, mc, h * 128:(h + 1) * 128],
                                    exs[q][:, js],
                                    start=(mc == 0), stop=(mc == KC - 1))
                            if RS_MODE == "par":
                                if mc == 0:
                                    nc.vector.tensor_copy(exacc[:], ex[:])
                                else:
                                    nc.vector.tensor_tensor(
                                        exacc[:], exacc[:], ex[:], ADD)
                        if RS_MODE == "mm":
                            for j in range(2):
                                js = slice(j * 512, (j + 1) * 512)
                                for q in range(4):
                                    nc.tensor.matmul(
                                        rs2[j][:], ones_full[:],
                                        exs[q][:, js],
                                        start=(blk == 0 and q == 0),
                                        stop=(blk == KC // 4 - 1 and q == 3))
                        elif RS_MODE == "ct":
                            for j in range(2):
                                js = slice(j * 512, (j + 1) * 512)
                                for q in range(4):
                                    nc.tensor.matmul(
                                        rs2[j][32 * q:32 * (q + 1), :],
                                        ones32_t[:], exs[q][:, js],
                                        start=(blk == 0),
                                        stop=(blk == KC // 4 - 1),
                                        tile_position=(0, 32 * q),
                                        skip_group_check=True)
                    # normalize: ot = pv / colsum -> DRAM ag_in (fp16)
                    if RS_MODE == "par":
                        rs_all = eac.tile([128, 1024], F32, name="rs_all",
                                          tag="exacc")
                        nc.gpsimd.partition_all_reduce(
                            rs_all[:], exacc[:], 128, bass_isa.ReduceOp.add)
                        rbc2 = ep.tile([128, 1024], F32, name="rbc2")
                        nc.vector.reciprocal(rbc2[:], rs_all[:])
                    for j, n in enumerate(ns):
                        ls = slice(n * 512, (n + 1) * 512)
                        js = slice(j * 512, (j + 1) * 512)
                        if RS_MODE == "par":
                            rbc = rbc2[:, js]
                        elif RS_MODE == "ct":
                            rsb = ep.tile([128, 512], F32R, name="rsb")
                            nc.vector.tensor_copy(rsb[:], rs2[j][:])
                            rall = scp.tile([128, 1024], F32, name="rall",
                                            tag="sc")
                            nc.tensor.matmul(rall[:, :512], comb_t[:],
                                             rsb[:], start=True, stop=True)
                            rbc_t = ep.tile([128, 512], F32, name="rbc")
                            nc.vector.reciprocal(rbc_t[:], rall[:, :512])
                            rbc = rbc_t[:]
                        else:
                            rbc_t = ep.tile([128, 512], F32, name="rbc")
                            nc.vector.reciprocal(rbc_t[:], rs2[j][:])
                            rbc = rbc_t[:]
                        ot = otp.tile([128, 512], F16, name="ot")
                        nc.vector.tensor_tensor(ot[:], pv2[j][:], rbc,
                                                MULT)
                        nc.sync.dma_start(ag_in_v[h][:, ls], ot[:])
                # AllGather: per-head (AG_CHUNKS=4, overlaps attention) or
                # one bulk gather after the last head (AG_CHUNKS=1)
                if AG_CHUNKS == 4:
                    if ag_local:
                        for g in range(GROUPS):
                            nc.gpsimd.dma_start(
                                ag_out[(h * GROUPS + g) * 128:
                                       (h * GROUPS + g + 1) * 128, :],
                                ag_in_v[h])
                    else:
                        nc.gpsimd.collective_compute(
                            "AllGather", mybir.AluOpType.bypass,
                            replica_groups=rg,
                            ins=[ag_in_v[h].opt()],
                            outs=[ag_out[h * 512:(h + 1) * 512, :].opt()])
            if AG_CHUNKS == 1:
                if ag_local:
                    for g in range(GROUPS):
                        nc.sync.dma_start(
                            ag_out[g * EL:(g + 1) * EL, :], ag_in[:])
                else:
                    nc.gpsimd.collective_compute(
                        "AllGather", mybir.AluOpType.bypass,
                        replica_groups=rg,
                        ins=[ag_in.opt()],
                        outs=[ag_out.opt()])

        # ---------------- out projection (og SBUF-resident) ----------------
        with tc.tile_pool(name="ogp", bufs=1) as ogp, \
             tc.tile_pool(name="pp", bufs=8, space="PSUM") as pp, \
             tc.tile_pool(name="yp", bufs=3) as yp:
            og = ogp.tile([128, KC, L], F16, name="og")
            for k in range(KC):
                nc.sync.dma_start(og[:, k, :], ag_out_v[k])
            for n in range(NT):
                ls = slice(n * 512, (n + 1) * 512)
                accs = [pp.tile([128, 512], F32, name=f"o_acc{m}",
                                tag="o_acc")
                        for m in range(MT)]
                for k in range(KC):
                    for m in range(MT):
                        nc.tensor.matmul(
                            accs[m][:],
                            w_t["wo"][:, k, m * 128:(m + 1) * 128],
                            og[:, k, ls],
                            start=(k == 0), stop=(k == KC - 1))
                for m in range(MT):
                    yt = yp.tile([128, 512], F32, name="yt")
                    nc.scalar.activation(yt[:], accs[m][:], IDENT,
                                         bias=bo_t[:, m:m + 1])
                    nc.sync.dma_start(y[m * 128:(m + 1) * 128, ls], yt[:])


def _to_f16(a):
    return np.asarray(a, np.float32).astype(np.float16)


def _to_bf16(a):
    import ml_dtypes
    return np.asarray(a, np.float32).astype(ml_dtypes.bfloat16)


def _stack_qk_w(Wr, Wi, g):
    """Transposed stacked projection weight [2048, 512] for head-group g."""
    hsl = slice(g * HL * D, (g + 1) * HL * D)
    top = np.concatenate([Wr[hsl].T, -Wi[hsl].T], axis=0)  # part=0 cols
    bot = np.concatenate([Wi[hsl].T, Wr[hsl].T], axis=0)   # part=1 cols
    return np.ascontiguousarray(
        np.stack([top.reshape(2 * E, HL, D), bot.reshape(2 * E, HL, D)],
                 axis=2).reshape(2 * E, EL))


def _pack_w(a):
    """[2048, F] -> [128, KC, F] with row k*128+p -> [p, k]."""
    return np.ascontiguousarray(
        a.reshape(KC, 128, a.shape[1]).transpose(1, 0, 2))


def _stack_bias(br, bi, g):
    hsl = slice(g * HL * D, (g + 1) * HL * D)
    s = np.stack([br[hsl].reshape(HL, D), bi[hsl].reshape(HL, D)],
                 axis=1).reshape(EL)
    return np.ascontiguousarray(s.reshape(MT, 128).T)  # [128, MT]


def prep_in_maps(inputs):
    f32 = np.float32
    xs = {}
    for b in range(B):
        for nm, xr, xi in (("xq", inputs["query_r"], inputs["query_i"]),
                           ("xk", inputs["key_r"], inputs["key_i"]),
                           ("xv", inputs["value_r"], inputs["value_i"])):
            stk = np.concatenate([np.asarray(xr[b]).T, np.asarray(xi[b]).T],
                                 axis=0).astype(np.float16)  # [2048, L]
            if nm == "xv":
                # [128, lc, k, 128]: (p, lc, k, c) = stk[k*128+p, lc*128+c]
                a = stk.reshape(KC, 128, KC, 128).transpose(1, 2, 0, 3)
            else:
                # [128, n, k, 512]: (p, n, k, c) = stk[k*128+p, n*512+c]
                a = stk.reshape(KC, 128, NT, 512).transpose(1, 2, 0, 3)
            xs[(nm, b)] = np.ascontiguousarray(a)

    # out-proj: full stacked weight [e''=2048, out_row=2048]
    WoT_r = np.asarray(inputs["Wo_r"]).T.astype(f32)
    WoT_i = np.asarray(inputs["Wo_i"]).T.astype(f32)
    top = np.concatenate([WoT_r, WoT_i], axis=1)    # part=0 rows
    bot = np.concatenate([-WoT_i, WoT_r], axis=1)   # part=1 rows
    inter = np.stack([top.reshape(H, D, 2 * E), bot.reshape(H, D, 2 * E)],
                     axis=1).reshape(2 * E, 2 * E)  # [(head,part,d), row]
    # per-head AllGather lays ag_out out as (h_local, rank) blocks; block
    # b = h_local*GROUPS + rank holds global head rank*HL + h_local.
    # the bulk AllGather (AG_CHUNKS=1) is rank-major: block b = global head b.
    if AG_CHUNKS == 4:
        perm = [(b % GROUPS) * HL + b // GROUPS for b in range(H)]
        inter = inter.reshape(H, 2 * D, 2 * E)[perm].reshape(2 * E, 2 * E)
    bo_cat = np.concatenate([np.asarray(inputs["bo_r"]),
                             np.asarray(inputs["bo_i"])]).astype(f32)

    in_maps = []
    for c in range(NCORES):
        b, g = divmod(c, GROUPS)
        hsl = slice(g * HL * D, (g + 1) * HL * D)
        bv_s = np.stack([np.asarray(inputs["bv_r"])[hsl].reshape(HL, D),
                         np.asarray(inputs["bv_i"])[hsl].reshape(HL, D)],
                        axis=1).reshape(EL).astype(f32)
        m = {
            "xq": xs[("xq", b)], "xk": xs[("xk", b)], "xv": xs[("xv", b)],
            "wq": _pack_w(_to_f16(_stack_qk_w(
                np.asarray(inputs["Wq_r"], f32),
                np.asarray(inputs["Wq_i"], f32), g))),
            "wk": _pack_w(_to_f16(_stack_qk_w(
                np.asarray(inputs["Wk_r"], f32),
                np.asarray(inputs["Wk_i"], f32), g))),
            "wv": _pack_w(_to_f16(_stack_qk_w(
                np.asarray(inputs["Wv_r"], f32),
                np.asarray(inputs["Wv_i"], f32), g))),
            "wo": _pack_w(_to_f16(np.ascontiguousarray(
                inter[:, g * EL:(g + 1) * EL]))),
            "ones": _to_bf16(np.ones((128, 128), f32)),
            "ones32": _to_bf16(np.ones((128, 32), f32)),
            "comb": np.full((128, 128), 1.0 / 32.0, f32),
            "bq": _stack_bias(np.asarray(inputs["bq_r"], f32),
                              np.asarray(inputs["bq_i"], f32), g),
            "bk": _stack_bias(np.asarray(inputs["bk_r"], f32),
                              np.asarray(inputs["bk_i"], f32), g),
            "bo": np.ascontiguousarray(
                bo_cat[g * EL:(g + 1) * EL].reshape(MT, 128).T),
            "bv": np.broadcast_to(bv_s, (128, EL)).copy(),
        }
        in_maps.append(m)
    return in_maps


def assemble(results):
    out = np.empty((2, B, L, E), np.float32)
    for b in range(B):
        ys = np.concatenate([results[b * GROUPS + g]["y"]
                             for g in range(GROUPS)], axis=0)  # [2048, L]
        out[0, b] = ys[:E].T
        out[1, b] = ys[E:].T
    return out


_NC_CACHE = {}


def get_nc(repeat: int = 1):
    if repeat not in _NC_CACHE:
        _NC_CACHE[repeat] = build_nc(repeat)
    return _NC_CACHE[repeat]


def make_runner(nc):
    """Build a reusable jitted SPMD executor for `nc` (compiles once).

    Mirrors concourse.bass2jax.run_bass_via_pjrt's multi-core path, but the
    jitted callable is constructed a single time so repeated invocations do
    not re-trigger the walrus/NEFF compile.
    """
    import jax
    from jax.experimental.shard_map import shard_map
    from jax.sharding import Mesh, PartitionSpec

    from concourse import bass2jax

    bass2jax.install_neuronx_cc_hook()
    assert nc.dbg_addr is None

    partition_name = (nc.partition_id_tensor.name
                      if nc.partition_id_tensor else None)
    in_names, out_names, out_avals, zero_outs = [], [], [], []
    for alloc in nc.m.functions[0].allocations:
        if not isinstance(alloc, mybir.MemoryLocationSet):
            continue
        name = alloc.memorylocations[0].name
        if alloc.kind == "ExternalInput":
            if name != partition_name:
                in_names.append(name)
        elif alloc.kind == "ExternalOutput":
            shape = tuple(alloc.tensor_shape)
            dtype = mybir.dt.np(alloc.dtype)
            out_names.append(name)
            out_avals.append(jax.core.ShapedArray(shape, dtype))
            zero_outs.append(np.zeros(shape, dtype))
    n_params = len(in_names)
    n_outs = len(out_avals)
    all_in_names = list(in_names) + list(out_names)
    if partition_name is not None:
        all_in_names.append(partition_name)

    def _body(*args):
        operands = list(args)
        if partition_name is not None:
            operands.append(bass2jax.partition_id_tensor())
        outs = bass2jax._bass_exec_p.bind(
            *operands,
            out_avals=tuple(out_avals),
            in_names=tuple(all_in_names),
            out_names=tuple(out_names),
            lowering_input_output_aliases=(),
            sim_require_finite=True,
            sim_require_nnan=True,
            nc=nc,
        )
        return tuple(outs)

    devices = jax.devices()[:NCORES]
    mesh = Mesh(np.asarray(devices), ("core",))
    specs_in = (PartitionSpec("core"),) * (n_params + n_outs)
    specs_out = (PartitionSpec("core"),) * n_outs
    donate = tuple(range(n_params, n_params + n_outs))
    sharded = jax.jit(
        shard_map(_body, mesh=mesh, in_specs=specs_in, out_specs=specs_out,
                  check_rep=False),
        donate_argnums=donate, keep_unused=True)

    def run(in_maps, device_inputs=None):
        if device_inputs is None:
            device_inputs = put_inputs(in_maps)
        concat_zeros = [
            np.zeros((NCORES * z.shape[0], *z.shape[1:]), z.dtype)
            for z in zero_outs]
        out_arrs = sharded(*device_inputs, *concat_zeros)
        jax.block_until_ready(out_arrs)
        return [
            {name: np.asarray(out_arrs[i]).reshape(
                NCORES, *out_avals[i].shape)[c]
             for i, name in enumerate(out_names)}
            for c in range(NCORES)]

    def put_inputs(in_maps):
        return [
            np.concatenate([np.asarray(in_maps[c][nm])
                            for c in range(NCORES)], axis=0)
            for nm in in_names]

    def put_device(in_maps):
        from jax.sharding import NamedSharding
        sh = NamedSharding(mesh, PartitionSpec("core"))
        arrs = [jax.device_put(a, sh) for a in put_inputs(in_maps)]
        jax.block_until_ready(arrs)
        return arrs

    run.put_inputs = put_inputs
    run.put_device = put_device
    return run


_RUNNER_CACHE = {}


def get_runner(repeat: int = 1):
    if repeat not in _RUNNER_CACHE:
        _RUNNER_CACHE[repeat] = make_runner(get_nc(repeat))
    return _RUNNER_CACHE[repeat]


def kernel(**inputs) -> np.ndarray:
    runner = get_runner(1)
    in_maps = prep_in_maps(inputs)
    results = runner(in_maps)
    return assemble(results)

